# revision 41
# baseline (speedup 1.0000x reference)
"""ContraFace loss kernel for 8 TRN2 NeuronCores.

Strategy: row-shard the [B, B] cosine matrix across 8 cores (B/8 = 1024 rows
per core). The device does the O(B^2 D) / O(B^2) work: the full cosine
matmul, same-label masking, per-row hardest-negative max, and per-row
sum(exp(S*cos)). Host supplies layout-prepped shards:
  - f2nt: L2-normalized f2, transposed to [D, B] (bf16) - the matmul moving
    operand, shared by all cores
  - f1t:  raw f1 shard transposed [D, BS] (bf16) - stationary operand
  - maskf: [128, MT, B] uint8 0/1 mask, 0 where label_col == label_row
    (zeroes same-label entries incl. the diagonal, matching the reference's
    cos=0 substitution)
  - srn1: [128, MT] fp32 = S / ||f1_row|| per-partition Exp scale

Device pipeline per core, per (m, g) tile of the [1024, 8192] block:
  - PE: 16 bf16 matmuls accumulate a [128, 2048] PSUM tile (1 cycle/row)
  - DVE tensor_tensor: vz = psum * mask (bf16 out)
  - DVE tensor_scalar on vz (4x bf16 mode) with accum_out op1=max -> row max
  - ACT Exp with per-partition scale srn1 and accum_out -> row sumexp
DMAs are split across queues (SP: f2 panels, GPSIMD: srn1/f1t) and ordered
so the PE starts ~3.3us in and never starves; the first two m-tiles run as
interleaved 512-wide pieces to match the DMA arrival rate, and the last
m-tile is split in half to shorten the end-of-kernel drain. Host does the
tiny O(B) combine in float64: positives, EMA margin m from (pos - neg),
cross-entropy mean.
"""

import sys

sys.path.insert(0, "/opt/trn_rl_repo")

import numpy as np
from contextlib import ExitStack

import ml_dtypes

from concourse import bass, bacc, tile
from concourse.bass_utils import run_bass_kernel_spmd
import concourse.mybir as mybir

dt = mybir.dt
Alu = mybir.AluOpType
Act = mybir.ActivationFunctionType

B, D = 8192, 512
NCORES = 8
BS = B // NCORES          # 1024 rows per core
MT = BS // 128            # 8 M-tiles per core
KC = D // 128             # 4 contraction chunks
GW = 2048                 # column group width (PSUM tile free size, 4 banks)
NG = B // GW              # 4 column groups
S = 64.0
EMA = 0.99

_prog_cache = {}


def _build_program():
    nc = bacc.Bacc(None)

    f1t_d = nc.declare_dram_parameter("f1t", [D, BS], dt.bfloat16, isOutput=False)
    f2nt_d = nc.declare_dram_parameter("f2nt", [D, B], dt.bfloat16, isOutput=False)
    mask_d = nc.declare_dram_parameter("maskf", [128, MT, B], dt.uint8, isOutput=False)
    srn1_d = nc.declare_dram_parameter("srn1", [128, MT], dt.float32, isOutput=False)

    # j-major stats layout over 8 column groups of 1024: column j*MT + m.
    # Extra columns 64-67 hold the 512-wide prologue pieces of (m0, m1).
    NJ = 2 * NG
    NSTAT = NJ * MT + 5
    mx_d = nc.declare_dram_parameter("mx", [128, NSTAT], dt.float32, isOutput=True)
    se_d = nc.declare_dram_parameter("se", [128, NSTAT], dt.float32, isOutput=True)

    f1t_v = f1t_d[:].rearrange("(c p) i -> p c i", p=128)
    f2nt_v = f2nt_d[:].rearrange("(c p) j -> p c j", p=128)

    with tile.TileContext(nc) as tc, ExitStack() as ctx:
        cst = ctx.enter_context(tc.tile_pool(name="cst", bufs=1))
        pan = ctx.enter_context(tc.tile_pool(name="pan", bufs=NG))
        mkp = ctx.enter_context(tc.tile_pool(name="mkp", bufs=NG))
        vzp = ctx.enter_context(tc.tile_pool(name="vzp", bufs=8))
        exq = ctx.enter_context(tc.tile_pool(name="exq", bufs=6))
        dmp = ctx.enter_context(tc.tile_pool(name="dmp", bufs=3))
        psm = ctx.enter_context(
            tc.tile_pool(name="psm", bufs=4, space=bass.MemorySpace.PSUM)
        )

        stats = cst.tile([128, NSTAT], dt.float32, tag="stats")
        sums = cst.tile([128, NSTAT], dt.float32, tag="sums")
        srn1 = cst.tile([128, MT], dt.float32, tag="srn1")
        f1t_sb = cst.tile([128, KC, BS], dt.bfloat16, tag="f1t")

        f2p = []
        mk = []
        for g in range(NG):
            f2p.append(pan.tile([128, KC, GW], dt.bfloat16, tag="f2p", name=f"f2p{g}"))
            mk.append(mkp.tile([128, MT, GW], dt.uint8, tag="mk", name=f"mk{g}"))

        # DMA priority order. Transfers serialize on the shared DMA device,
        # but descriptor-generation overhead is per-queue - so the prologue
        # spreads across three queues: SP carries the f2 panels (PE-critical),
        # ACT carries the masks, GPSIMD carries srn1/f1t. m0 and m1
        # interleave on each f2p[0] quarter so the PE matches the DMA rate.
        nc.gpsimd.dma_start(f1t_sb[:, :, 0:256], f1t_v[:, :, 0:256])
        nc.gpsimd.dma_start(f1t_sb[:, :, 256:512], f1t_v[:, :, 256:512])
        nc.gpsimd.dma_start(srn1[:], srn1_d[:])
        nc.gpsimd.dma_start(f1t_sb[:, :, 512:BS], f1t_v[:, :, 512:BS])
        nc.sync.dma_start(f2p[0][:, :, 0:512], f2nt_v[:, :, 0:512])
        nc.sync.dma_start(mk[0][:, 0:2, 0:512], mask_d[:, 0:2, 0:512])
        nc.sync.dma_start(f2p[0][:, :, 512:1024], f2nt_v[:, :, 512:1024])
        nc.sync.dma_start(mk[0][:, 0:2, 512:1024], mask_d[:, 0:2, 512:1024])
        nc.sync.dma_start(f2p[0][:, :, 1024:1536], f2nt_v[:, :, 1024:1536])
        nc.sync.dma_start(mk[0][:, 0:2, 1024:1536], mask_d[:, 0:2, 1024:1536])
        nc.sync.dma_start(f2p[0][:, :, 1536:2048], f2nt_v[:, :, 1536:2048])
        nc.sync.dma_start(mk[0][:, 0:2, 1536:2048], mask_d[:, 0:2, 1536:2048])
        nc.sync.dma_start(mk[0][:, 2:5, :], mask_d[:, 2:5, 0:GW])
        nc.sync.dma_start(mk[0][:, 5:MT, :], mask_d[:, 5:MT, 0:GW])
        nc.sync.dma_start(f2p[1][:], f2nt_v[:, :, GW : 2 * GW])
        nc.sync.dma_start(mk[1][:], mask_d[:, :, GW : 2 * GW])
        for g in range(2, NG):
            nc.sync.dma_start(f2p[g][:], f2nt_v[:, :, g * GW : (g + 1) * GW])
            nc.sync.dma_start(mk[g][:], mask_d[:, :, g * GW : (g + 1) * GW])

        # ---- Main loop: matmul -> mask (DVE tt) -> max (DVE ts) -> exp (ACT)
        def emit_group(g, m, col0, width, stat_col, split_tt=False):
            acc = psm.tile([128, width], dt.float32, tag="acc", name="acc")
            for s0 in range(0, width, 512):
                sw = min(512, width - s0)
                for c in range(KC):
                    nc.tensor.matmul(
                        acc[:, s0 : s0 + sw],
                        f1t_sb[:, c, m * 128 : (m + 1) * 128],
                        f2p[g][:, c, col0 + s0 : col0 + s0 + sw],
                        start=(c == 0),
                        stop=(c == KC - 1),
                    )
            vz = vzp.tile([128, width], dt.bfloat16, tag="vz", name="vz")
            if split_tt:
                # two half-width mask passes: the first starts as soon as the
                # first half of the matmuls lands (subtile deps), freeing the
                # PSUM slot earlier in the drain region
                h = width // 2
                nc.vector.tensor_tensor(
                    out=vz[:, 0:h], in0=acc[:, 0:h],
                    in1=mk[g][:, m, col0 : col0 + h], op=Alu.mult,
                )
                nc.vector.tensor_tensor(
                    out=vz[:, h:width], in0=acc[:, h:width],
                    in1=mk[g][:, m, col0 + h : col0 + width], op=Alu.mult,
                )
            else:
                nc.vector.tensor_tensor(
                    out=vz[:], in0=acc[:], in1=mk[g][:, m, col0 : col0 + width],
                    op=Alu.mult,
                )
            dum = dmp.tile([128, width], dt.bfloat16, tag="dum", name="dum")
            nc.vector.tensor_scalar(
                out=dum[:], in0=vz[:], scalar1=1.0, scalar2=None,
                op0=Alu.mult, op1=Alu.max,
                accum_out=stats[:, stat_col : stat_col + 1],
            )
            ex = exq.tile([128, width], dt.bfloat16, tag="ex", name="ex")
            nc.scalar.activation(
                ex[:],
                vz[:],
                Act.Exp,
                bias=0.0,
                scale=srn1[:, m : m + 1],
                accum_out=sums[:, stat_col : stat_col + 1],
            )

        # prologue: m0/m1 on the first panel in interleaved 512-wide
        # pieces, pacing the PE to the DMA arrival rate of the f2p[0]
        # quarters. piece p covers panel cols [512p, 512p+512) = group j=p//2;
        # even pieces use the main (j, m) stat col, odd pieces the extras.
        for piece in range(4):
            for m in range(2):
                if piece % 2 == 0:
                    col = (piece // 2) * MT + m
                else:
                    col = NJ * MT + (piece // 2) * 2 + m
                emit_group(0, m, piece * 512, 512, col)
        # main loop: 1024-wide groups j over panels j//2; the 4-deep PSUM
        # ring keeps the PE ahead of the DVE drain at all times
        for j in range(NJ):
            for m in range(MT):
                if j <= 1 and m < 2:
                    continue
                if j == NJ - 1 and m == MT - 1:
                    # final tile as 2x512 so the tail drain chain is short
                    emit_group(j // 2, m, (j % 2) * 1024, 512, j * MT + m)
                    emit_group(j // 2, m, (j % 2) * 1024 + 512, 512,
                               NJ * MT + 4)
                else:
                    emit_group(j // 2, m, (j % 2) * 1024, 1024, j * MT + m)
            if j % 2 == 1 and j < NJ - 1:
                # write back finished panel stats (2 groups = 16 cols)
                lo, hi = (j - 1) * MT, (j + 1) * MT
                nc.gpsimd.dma_start(mx_d[:, lo:hi], stats[:, lo:hi])
                nc.sync.dma_start(se_d[:, lo:hi], sums[:, lo:hi])
        # final writeback: last panel's 16 cols plus the 4 extras
        lo = (NJ - 2) * MT
        nc.gpsimd.dma_start(mx_d[:, lo:], stats[:, lo:])
        nc.sync.dma_start(se_d[:, lo:], sums[:, lo:])

    if not nc.is_finalized():
        nc.finalize()
    return nc


def _get_program():
    if "nc" not in _prog_cache:
        _prog_cache["nc"] = _build_program()
    return _prog_cache["nc"]


def _host_inputs(feature1, feature2, label):
    bf16 = ml_dtypes.bfloat16
    f1 = np.asarray(feature1, dtype=np.float32)
    f2 = np.asarray(feature2, dtype=np.float32)
    lab = np.asarray(label)

    f2n = f2 / np.linalg.norm(f2, axis=1, keepdims=True)
    f2nt = np.ascontiguousarray(f2n.T.astype(bf16))
    rn1_all = 1.0 / np.linalg.norm(f1.astype(np.float64), axis=1)

    in_maps = []
    for c in range(NCORES):
        sl = slice(c * BS, (c + 1) * BS)
        f1s = f1[sl]
        same = lab[sl, None] == lab[None, :]                  # [BS, B]
        maskf = np.ascontiguousarray(
            (~same).astype(np.uint8).reshape(MT, 128, B).transpose(1, 0, 2)
        )
        srn1 = np.ascontiguousarray(
            (S * rn1_all[sl]).reshape(MT, 128).T.astype(np.float32)
        )
        in_maps.append(
            dict(
                f1t=np.ascontiguousarray(f1s.T.astype(bf16)),
                f2nt=f2nt,
                maskf=maskf,
                srn1=srn1,
            )
        )
    return in_maps


def kernel(feature1, feature2, label, _want_results=False, _trace=False):
    f1 = np.asarray(feature1, dtype=np.float32)
    f2 = np.asarray(feature2, dtype=np.float32)
    in_maps = _host_inputs(f1, f2, label)

    nc = _get_program()
    kw = {}
    if _trace:
        kw = dict(trace=True)
    out = run_bass_kernel_spmd(nc, in_maps, list(range(NCORES)), **kw)
    res = out.results

    # host O(B) combine in float64
    f1_64 = f1.astype(np.float64)
    f2_64 = f2.astype(np.float64)
    rn1 = 1.0 / np.linalg.norm(f1_64, axis=1)                 # [B]
    rn2 = 1.0 / np.linalg.norm(f2_64, axis=1)
    pos = np.clip(np.einsum("ij,ij->i", f1_64, f2_64) * rn1 * rn2, -1.0, 1.0)

    neg = np.empty(B, dtype=np.float64)
    sumoff = np.empty(B, dtype=np.float64)
    for c in range(NCORES):
        r = res[c]
        sl = slice(c * BS, (c + 1) * BS)
        NJ = 2 * NG
        mx = r["mx"].astype(np.float64)                       # [128, NJ*MT+4]
        se = r["se"].astype(np.float64)
        mxm = mx[:, : NJ * MT].reshape(128, NJ, MT).max(axis=1)   # [128, MT]
        sem = se[:, : NJ * MT].reshape(128, NJ, MT).sum(axis=1)
        # fold prologue-piece extras: 64,65 -> m0; 66,67 -> m1
        E = NJ * MT
        mxm[:, 0] = np.maximum(mxm[:, 0], np.maximum(mx[:, E], mx[:, E + 2]))
        sem[:, 0] += se[:, E] + se[:, E + 2]
        mxm[:, 1] = np.maximum(mxm[:, 1], np.maximum(mx[:, E + 1], mx[:, E + 3]))
        sem[:, 1] += se[:, E + 1] + se[:, E + 3]
        mxm[:, MT - 1] = np.maximum(mxm[:, MT - 1], mx[:, E + 4])
        sem[:, MT - 1] += se[:, E + 4]
        neg[sl] = mxm.T.reshape(BS) * rn1[sl]                 # raw-dot max * rn1
        sumoff[sl] = sem.T.reshape(BS) - 1.0

    m = EMA * np.mean(pos - neg)
    z = S * (pos - m)
    loss = np.mean(np.log(sumoff + np.exp(z)) - z)
    out_val = np.float32(loss)
    if _want_results:
        return out_val, out
    return out_val
